# revision 13
# baseline (speedup 1.0000x reference)
"""ALiBi attention (B=2, S=2048, C=1024, H=16) on 8 trn2 NeuronCores.

Sharding: head-parallel. Core c owns heads (c, c+8) for both batches:
  - in_proj computed per-core only for its 6 head-slices (q,k,v x 2 heads),
    directly in transposed [channel, token] layout (x is host-transposed).
  - scores are computed transposed (S^T[j,i] = k_j . q_i) so softmax j-sums
    come from a ones-column augmented onto v, and the probability matrix is
    never transposed.
  - ALiBi bias min(slope*(i-j), 8) is injected into the score PSUM with an
    identity matmul against a host-precomputed shifted bias table; tiles where
    the bias is saturated at +8 skip the inject (the +8 cancels against the
    exp's -8 range shift), and far-future tiles with negligible probability
    mass are skipped entirely. Both classifications depend only on the head
    SLOT (slot 0 = heads 0..7, slot 1 = heads 8..15), so the single SPMD
    program stays valid on every core.
  - k stationaries are zero-padded to K=128 per head (the other head's rows
    are 0, killing its q rows in the shared moving operand): mixed K=64/K=128
    f32r matmul streams reconfigure the PE array and run ~3x slower.
  - out_proj is row-parallel: each core emits a partial y; the host sums the
    8 partials and adds out_proj_bias (the "all-reduce").
"""
import functools
import math
import sys

sys.path.insert(0, "/opt/trn_rl_repo")

import numpy as np

B, S, C, H, D = 2, 2048, 1024, 16, 64
TOK = B * S
NCORE = 8
MAX_BIAS = 8.0
BTW = 2 * S - 128       # shifted bias-table width (full, for slot-1 heads)
BT0_OFF = 384           # slot-0 table column offset (unfolded tiles only)
BT0_W = 2816            # slot-0 table width
SCALE = float(D) ** -0.5
SKIP_J_MINUS_I = 1483   # skip tile if j0 - i0 >= this (slot 0 only)
FOLD_I_MINUS_J = 255    # inject-free tile if i0 - j0 >= this (slot 0 only)


def _slopes() -> np.ndarray:
    start = 2.0 ** (-(2.0 ** (-(math.log2(H) - 3))))
    return np.array([start * start**i for i in range(H)], dtype=np.float32)


@functools.lru_cache(maxsize=1)
def _program():
    import concourse.mybir as mybir
    import concourse.tile as tile
    from concourse import bacc
    from concourse.masks import make_identity

    F32 = mybir.dt.float32
    F32R = mybir.dt.float32r
    F16 = mybir.dt.float16
    BF16 = mybir.dt.bfloat16
    U32 = mybir.dt.uint32
    Exp = mybir.ActivationFunctionType.Exp
    MUL = mybir.AluOpType.mult

    nc = bacc.Bacc("TRN2", target_bir_lowering=False, debug=False)

    xt = nc.dram_tensor("xt", [C, TOK], F32R, kind="ExternalInput").ap()
    wqkvt = nc.dram_tensor("wqkvt", [C, 384], F32R, kind="ExternalInput").ap()
    bqkv = nc.dram_tensor("bqkv", [128, 3], F32, kind="ExternalInput").ap()
    bt = nc.dram_tensor("bt", [2, 128, BTW], F32R, kind="ExternalInput").ap()
    wot = nc.dram_tensor("wot", [128, C], F32R, kind="ExternalInput").ap()
    y = nc.dram_tensor("y", [TOK, C], F32, kind="ExternalOutput").ap()

    with tile.TileContext(nc) as tc:
        with tc.tile_pool(name="const", bufs=1) as cpool, \
             tc.tile_pool(name="wpool", bufs=1) as wpool, \
             tc.tile_pool(name="qkvp", bufs=1) as qkvp, \
             tc.tile_pool(name="xin", bufs=2) as xpool, \
             tc.tile_pool(name="probs", bufs=2) as ppool, \
             tc.tile_pool(name="work", bufs=2) as wk, \
             tc.tile_pool(name="ps", bufs=2, space="PSUM") as ps:

            ident = cpool.tile([128, 128], F32, name="ident")
            make_identity(nc, ident[:])
            identr = cpool.tile([128, 128], F32R, name="identr")
            nc.vector.tensor_copy(identr[:], ident[:])
            neg8 = cpool.tile([128, 1], F32, name="neg8")
            nc.vector.memset(neg8[:], -MAX_BIAS)
            zero0 = cpool.tile([128, 1], F32, name="zero0")
            nc.vector.memset(zero0[:], 0.0)
            heat = cpool.tile([128, 128], BF16, name="heat")
            nc.vector.tensor_copy(heat[:], ident[:])

            wq_sb = wpool.tile([128, 8, 384], F32R, name="wq_sb")
            nc.sync.dma_start(wq_sb[:],
                              wqkvt.rearrange("(co p) n -> p co n", p=128))
            bq_sb = wpool.tile([128, 3], F32, name="bq_sb")
            nc.sync.dma_start(bq_sb[:], bqkv)
            btab1 = wpool.tile([128, BTW], F32R, name="btab1")
            btab0 = wpool.tile([128, BT0_W], F32R, name="btab0")
            wo_sb = wpool.tile([128, C], F32R, name="wo_sb")

            def load_tables():
                nc.sync.dma_start(btab1[:],
                                  bt.rearrange("h p c -> p h c")[:, 1, :])
                nc.sync.dma_start(
                    btab0[:],
                    bt.rearrange("h p c -> p h c")[:, 0,
                                                   BT0_OFF:BT0_OFF + BT0_W])
                nc.sync.dma_start(wo_sb[:], wot)

            qkvT = qkvp.tile([128, 3, TOK], F32R, name="qkvT")
            kpadB = qkvp.tile([128, TOK], F32R, name="kpadB")
            nc.vector.memset(qkvT[64:128, 1, :].bitcast(U32), 0)
            nc.vector.memset(kpadB[0:64, :].bitcast(U32), 0)
            v_nat = qkvp.tile([128, 32, 2, 65], F16, name="v_nat")
            nc.vector.memset(v_nat[:, :, :, 64:65], 1.0)
            oT = qkvp.tile([128, TOK], F32R, name="oT")

            xt_r = xt.rearrange("(co p) t -> p co t", p=128)
            y_r = y.rearrange("(tb p) c -> tb p c", p=128)

            def in_proj(bb):
                for tb in range(4 * bb, 4 * bb + 4):
                    xtile = xpool.tile([128, 8, 512], F32R, name=f"xt{tb}",
                                       tag="xtile")
                    for cb in range(8):
                        nc.sync.dma_start(
                            xtile[:, cb:cb + 1, :],
                            xt_r[:, cb:cb + 1, tb * 512:(tb + 1) * 512])
                    for chb in range(3):
                        pin = ps.tile([128, 512], F32, name=f"pin{tb}_{chb}",
                                      tag="sc")
                        for cb in range(8):
                            nc.tensor.matmul(
                                pin[:],
                                wq_sb[:, cb, chb * 128:(chb + 1) * 128],
                                xtile[:, cb, :],
                                start=(cb == 0), stop=(cb == 7))
                        ts = slice(tb * 512, (tb + 1) * 512)
                        if chb == 1:
                            nc.vector.tensor_scalar_add(
                                qkvT[0:64, 1, ts], pin[0:64], bq_sb[0:64, 1:2])
                            nc.vector.tensor_scalar_add(
                                kpadB[64:128, ts], pin[64:128],
                                bq_sb[64:128, 1:2])
                        else:
                            nc.vector.tensor_scalar_add(
                                qkvT[:, chb, ts], pin[:], bq_sb[:, chb:chb + 1])

            def v_transpose(bb):
                for t32 in range(16 * bb, 16 * bb + 16):
                    pv = ps.tile([128, 128], F32, name=f"pv{t32}", tag="sc")
                    nc.tensor.transpose(
                        pv[:],
                        qkvT[:, 2, t32 * 128:(t32 + 1) * 128].bitcast(F32),
                        ident[:])
                    for hh in range(2):
                        nc.vector.tensor_copy(v_nat[:, t32, hh, 0:64],
                                              pv[:, hh * 64:hh * 64 + 64])

            def attn_iter(b, ih, hh):
                hb = hh * 64
                i0 = ih * 1024
                it = f"{b}{ih}{hh}"
                js = [j for j in range(16)
                      if not (hh == 0 and j * 128 - i0 >= SKIP_J_MINUS_I)]
                pacc = ps.tile([65, 1024], F32, name=f"pa{it}", tag="acc",
                               bufs=1)
                pending = None
                for idx, j in enumerate(js):
                    j0 = j * 128
                    fold = hh == 0 and i0 - j0 >= FOLD_I_MINUS_J
                    pS = ps.tile([128, 1024], F32, name=f"pS{it}_{j}", tag="sc")
                    if hh == 0:
                        kT = qkvT[:, 1, b * 2048 + j0: b * 2048 + j0 + 128]
                    else:
                        kT = kpadB[:, b * 2048 + j0: b * 2048 + j0 + 128]
                    dve_inj = hh == 1 and not fold
                    for iq in range(2):
                        ii = i0 + iq * 512
                        sl = pS[:, iq * 512:(iq + 1) * 512]
                        qT = qkvT[:, 0, b * 2048 + ii: b * 2048 + ii + 512]
                        nc.tensor.matmul(sl, kT, qT, start=True,
                                         stop=fold or dve_inj)
                        if not fold and not dve_inj:
                            c0 = ii - j0 + (S - 128)
                            rhs = btab0[:, c0 - BT0_OFF:c0 - BT0_OFF + 512]
                            nc.tensor.matmul(sl, identr[:], rhs,
                                             start=False, stop=True)
                    if dve_inj:
                        c0 = i0 - j0 + (S - 128)
                        nc.vector.tensor_tensor(pS[:], pS[:],
                                                btab1[:, c0:c0 + 1024],
                                                mybir.AluOpType.add)
                    pb = ppool.tile([128, 1024], F16, name=f"pb{it}_{j}",
                                    tag="pb")
                    nc.scalar.activation(pb[:], pS[:], Exp,
                                         bias=(zero0 if fold else neg8)[:, 0:1],
                                         scale=1.0)
                    if pending is not None:
                        pvb, pvj, first = pending
                        for iq in range(2):
                            nc.tensor.matmul(pacc[:, iq * 512:(iq + 1) * 512],
                                             v_nat[:, b * 16 + pvj, hh, :],
                                             pvb[:, iq * 512:(iq + 1) * 512],
                                             start=first, stop=False)
                    pending = (pb, j, idx == 0)
                pvb, pvj, first = pending
                for iq in range(2):
                    nc.tensor.matmul(pacc[:, iq * 512:(iq + 1) * 512],
                                     v_nat[:, b * 16 + pvj, hh, :],
                                     pvb[:, iq * 512:(iq + 1) * 512],
                                     start=first, stop=True)
                # normalization: oT = pacc[0:64] * (1/rowsum).
                # reciprocal runs in [8,128] layout (cheap); row<->col reshapes
                # ride on DMA; the broadcast runs on the idle GpSimd engine.
                sumr = wk.tile([1, 1024], F32, name=f"sr{it}", tag="sumr",
                               bufs=1)
                nc.vector.tensor_copy(sumr[:], pacc[64:65, :])
                sumc = wk.tile([8, 128], F32, name=f"sc{it}", tag="sumc")
                nc.sync.dma_start(sumc[:],
                                  sumr[:].rearrange("o (p a) -> o p a", a=128))
                inv8 = wk.tile([8, 128], F32, name=f"i8{it}", tag="inv8")
                nc.vector.reciprocal(inv8[:], sumc[:])
                invr = wk.tile([1, 1024], F32, name=f"iv{it}", tag="invr",
                               bufs=1)
                nc.sync.dma_start(invr[:].rearrange("o (p a) -> o p a", a=128),
                                  inv8[:])
                invbc = wk.tile([128, 1024], F32, name=f"ib{it}", tag="invbc",
                                bufs=1)
                nc.gpsimd.partition_broadcast(invbc[:], invr[:], channels=128)
                osl = oT[hb:hb + 64, b * 2048 + i0: b * 2048 + i0 + 1024]
                with nc.allow_low_precision(reason="f32r out"):
                    nc.vector.tensor_copy(osl, pacc[0:64, :])
                    nc.vector.tensor_tensor(osl, osl, invbc[hb:hb + 64, :], MUL)

            def out_proj(b, ih):
                for tloc in range(8):
                    tb = b * 16 + ih * 8 + tloc
                    for cq in range(2):
                        py_ = ps.tile([128, 512], F32, name=f"py{tb}_{cq}",
                                      tag="py")
                        nc.tensor.matmul(py_[:],
                                         oT[:, tb * 128:(tb + 1) * 128],
                                         wo_sb[:, cq * 512:(cq + 1) * 512],
                                         start=True, stop=True)
                        ytile = wk.tile([128, 512], F32, name=f"yt{tb}_{cq}",
                                        tag="ytile")
                        if cq == 0:
                            nc.vector.tensor_copy(ytile[:], py_[:])
                        else:
                            nc.scalar.copy(ytile[:], py_[:])
                        nc.sync.dma_start(y_r[tb][:, cq * 512:(cq + 1) * 512],
                                          ytile[:])

            in_proj(0)
            load_tables()
            v_transpose(0)
            attn_iter(0, 0, 0)
            attn_iter(0, 0, 1)
            attn_iter(0, 1, 0)
            attn_iter(0, 1, 1)
            in_proj(1)
            v_transpose(1)
            attn_iter(1, 0, 0)
            out_proj(0, 0)
            attn_iter(1, 0, 1)
            out_proj(0, 1)
            attn_iter(1, 1, 0)
            out_proj(1, 0)
            attn_iter(1, 1, 1)
            out_proj(1, 1)

    nc.compile()
    return nc


def _make_inmaps(x, in_proj_weight, in_proj_bias, out_proj_weight):
    slopes = _slopes()
    xT = np.ascontiguousarray(
        x.reshape(TOK, C).T.astype(np.float32))  # [C, TOK]

    in_maps = []
    p = np.arange(128, dtype=np.float64)[:, None]
    cc = np.arange(BTW, dtype=np.float64)[None, :]
    for c in range(NCORE):
        heads = (c, c + 8)
        rows = []
        for sec in range(3):  # q, k, v
            for h in heads:
                rows.extend(range(sec * C + h * D, sec * C + (h + 1) * D))
        rows = np.array(rows)
        wq = in_proj_weight[rows, :].astype(np.float32).copy()
        bq = in_proj_bias[rows].astype(np.float32).copy()
        wq[:128] *= SCALE  # fold q scaling
        bq[:128] *= SCALE
        wqkvt = np.ascontiguousarray(wq.T)  # [C, 384]
        bqkv = np.ascontiguousarray(bq.reshape(3, 128).T)  # [128, 3]

        btarr = np.empty((2, 128, BTW), dtype=np.float32)
        for hh, h in enumerate(heads):
            btarr[hh] = np.minimum(
                float(slopes[h]) * (cc - (S - 128) - p), float(MAX_BIAS)
            ).astype(np.float32)

        ocols = np.array(
            [heads[0] * D + d for d in range(D)]
            + [heads[1] * D + d for d in range(D)]
        )
        wotr = np.ascontiguousarray(
            out_proj_weight[:, ocols].T.astype(np.float32))  # [128, C]

        in_maps.append({
            "xt": xT,
            "wqkvt": wqkvt,
            "bqkv": bqkv,
            "bt": btarr,
            "wot": wotr,
        })
    return in_maps


def run(inputs: dict, trace: bool = False):
    from concourse.bass_utils import run_bass_kernel_spmd

    nc = _program()
    in_maps = _make_inmaps(
        np.asarray(inputs["x"]),
        np.asarray(inputs["in_proj_weight"]),
        np.asarray(inputs["in_proj_bias"]),
        np.asarray(inputs["out_proj_weight"]),
    )
    res = run_bass_kernel_spmd(nc, in_maps, list(range(NCORE)), trace=trace)
    acc = np.zeros((TOK, C), dtype=np.float64)
    for r in res.results:
        acc += r["y"].astype(np.float64)
    acc += np.asarray(inputs["out_proj_bias"]).astype(np.float64)[None, :]
    out = acc.astype(np.float32).reshape(B, S, C)
    return out, res


def kernel(**inputs) -> np.ndarray:
    return run(inputs, trace=False)[0]


# revision 14
# speedup vs baseline: 1.0979x; 1.0979x over previous
"""ALiBi attention (B=2, S=2048, C=1024, H=16) on 8 trn2 NeuronCores.

Sharding: head-parallel. Core c owns heads (c, c+8) for both batches:
  - in_proj computed per-core only for its 6 head-slices (q,k,v x 2 heads),
    directly in transposed [channel, token] layout (x is host-transposed).
  - scores are computed transposed (S^T[j,i] = k_j . q_i) so softmax j-sums
    come from a ones-column augmented onto v, and the probability matrix is
    never transposed.
  - ALiBi bias min(slope*(i-j), 8) is injected into the score PSUM with an
    identity matmul against a host-precomputed shifted bias table; tiles where
    the bias is saturated at +8 skip the inject (the +8 cancels against the
    exp's -8 range shift), and far-future tiles with negligible probability
    mass are skipped entirely. Both classifications depend only on the head
    SLOT (slot 0 = heads 0..7, slot 1 = heads 8..15), so the single SPMD
    program stays valid on every core.
  - k stationaries are zero-padded to K=128 per head (the other head's rows
    are 0, killing its q rows in the shared moving operand): mixed K=64/K=128
    f32r matmul streams reconfigure the PE array and run ~3x slower.
  - out_proj is row-parallel: each core emits a partial y; the host sums the
    8 partials and adds out_proj_bias (the "all-reduce").
"""
import functools
import math
import sys

sys.path.insert(0, "/opt/trn_rl_repo")

import numpy as np

B, S, C, H, D = 2, 2048, 1024, 16, 64
TOK = B * S
NCORE = 8
MAX_BIAS = 8.0
BTW = 2 * S - 128       # shifted bias-table width (full, for slot-1 heads)
BT0_OFF = 384           # slot-0 table column offset (unfolded tiles only)
BT0_W = 2816            # slot-0 table width
SCALE = float(D) ** -0.5
SKIP_J_MINUS_I = 1483   # skip tile if j0 - i0 >= this (slot 0 only)
FOLD_I_MINUS_J = 255    # inject-free tile if i0 - j0 >= this (slot 0 only)


def _slopes() -> np.ndarray:
    start = 2.0 ** (-(2.0 ** (-(math.log2(H) - 3))))
    return np.array([start * start**i for i in range(H)], dtype=np.float32)


@functools.lru_cache(maxsize=1)
def _program():
    import concourse.mybir as mybir
    import concourse.tile as tile
    from concourse import bacc
    from concourse.masks import make_identity

    F32 = mybir.dt.float32
    F32R = mybir.dt.float32r
    F16 = mybir.dt.float16
    BF16 = mybir.dt.bfloat16
    U32 = mybir.dt.uint32
    Exp = mybir.ActivationFunctionType.Exp
    MUL = mybir.AluOpType.mult

    nc = bacc.Bacc("TRN2", target_bir_lowering=False, debug=False)

    xt = nc.dram_tensor("xt", [C, TOK], F32R, kind="ExternalInput").ap()
    wqkvt = nc.dram_tensor("wqkvt", [C, 384], F32R, kind="ExternalInput").ap()
    bqkv = nc.dram_tensor("bqkv", [128, 3], F32, kind="ExternalInput").ap()
    bt = nc.dram_tensor("bt", [2, 128, BTW], F32R, kind="ExternalInput").ap()
    wot = nc.dram_tensor("wot", [128, C], F32R, kind="ExternalInput").ap()
    y = nc.dram_tensor("y", [TOK, C], F32, kind="ExternalOutput").ap()

    with tile.TileContext(nc) as tc:
        with tc.tile_pool(name="const", bufs=1) as cpool, \
             tc.tile_pool(name="wpool", bufs=1) as wpool, \
             tc.tile_pool(name="qkvp", bufs=1) as qkvp, \
             tc.tile_pool(name="xin", bufs=2) as xpool, \
             tc.tile_pool(name="probs", bufs=2) as ppool, \
             tc.tile_pool(name="work", bufs=2) as wk, \
             tc.tile_pool(name="ps", bufs=2, space="PSUM") as ps:

            ident = cpool.tile([128, 128], F32, name="ident")
            make_identity(nc, ident[:])
            identr = cpool.tile([128, 128], F32R, name="identr")
            nc.vector.tensor_copy(identr[:], ident[:])
            neg8 = cpool.tile([128, 1], F32, name="neg8")
            nc.vector.memset(neg8[:], -MAX_BIAS)
            zero0 = cpool.tile([128, 1], F32, name="zero0")
            nc.vector.memset(zero0[:], 0.0)
            heat = cpool.tile([128, 128], BF16, name="heat")
            nc.vector.tensor_copy(heat[:], ident[:])

            wq_sb = wpool.tile([128, 8, 384], F32R, name="wq_sb")
            nc.sync.dma_start(wq_sb[:],
                              wqkvt.rearrange("(co p) n -> p co n", p=128))
            bq_sb = wpool.tile([128, 3], F32, name="bq_sb")
            nc.sync.dma_start(bq_sb[:], bqkv)
            btab1 = wpool.tile([128, BTW], F32R, name="btab1")
            btab0 = wpool.tile([128, BT0_W], F32R, name="btab0")
            wo_sb = wpool.tile([128, C], F32R, name="wo_sb")

            def load_tables():
                nc.sync.dma_start(btab1[:],
                                  bt.rearrange("h p c -> p h c")[:, 1, :])
                nc.sync.dma_start(
                    btab0[:],
                    bt.rearrange("h p c -> p h c")[:, 0,
                                                   BT0_OFF:BT0_OFF + BT0_W])
                nc.sync.dma_start(wo_sb[:], wot)

            qkvT = qkvp.tile([128, 3, TOK], F32R, name="qkvT")
            kpadB = qkvp.tile([128, TOK], F32R, name="kpadB")
            nc.vector.memset(qkvT[64:128, 1, :].bitcast(U32), 0)
            nc.vector.memset(kpadB[0:64, :].bitcast(U32), 0)
            v_nat = qkvp.tile([128, 32, 2, 65], F16, name="v_nat")
            nc.vector.memset(v_nat[:, :, :, 64:65], 1.0)
            oT = qkvp.tile([128, TOK], F32R, name="oT")

            xt_r = xt.rearrange("(co p) t -> p co t", p=128)
            y_r = y.rearrange("(tb p) c -> tb p c", p=128)

            def in_proj(bb):
                for tb in range(4 * bb, 4 * bb + 4):
                    xtile = xpool.tile([128, 8, 512], F32R, name=f"xt{tb}",
                                       tag="xtile")
                    for cb in range(8):
                        nc.sync.dma_start(
                            xtile[:, cb:cb + 1, :],
                            xt_r[:, cb:cb + 1, tb * 512:(tb + 1) * 512])
                    for chb in range(3):
                        pin = ps.tile([128, 512], F32, name=f"pin{tb}_{chb}",
                                      tag="sc")
                        for cb in range(8):
                            nc.tensor.matmul(
                                pin[:],
                                wq_sb[:, cb, chb * 128:(chb + 1) * 128],
                                xtile[:, cb, :],
                                start=(cb == 0), stop=(cb == 7))
                        ts = slice(tb * 512, (tb + 1) * 512)
                        if chb == 1:
                            nc.vector.tensor_scalar_add(
                                qkvT[0:64, 1, ts], pin[0:64], bq_sb[0:64, 1:2])
                            nc.vector.tensor_scalar_add(
                                kpadB[64:128, ts], pin[64:128],
                                bq_sb[64:128, 1:2])
                        else:
                            nc.vector.tensor_scalar_add(
                                qkvT[:, chb, ts], pin[:], bq_sb[:, chb:chb + 1])

            def v_transpose(bb):
                for t32 in range(16 * bb, 16 * bb + 16):
                    pv = ps.tile([128, 128], F32, name=f"pv{t32}", tag="sc")
                    nc.tensor.transpose(
                        pv[:],
                        qkvT[:, 2, t32 * 128:(t32 + 1) * 128].bitcast(F32),
                        ident[:])
                    for hh in range(2):
                        nc.vector.tensor_copy(v_nat[:, t32, hh, 0:64],
                                              pv[:, hh * 64:hh * 64 + 64])

            def attn_iter(b, ih, hh):
                hb = hh * 64
                i0 = ih * 1024
                it = f"{b}{ih}{hh}"
                js = [j for j in range(16)
                      if not (hh == 0 and j * 128 - i0 >= SKIP_J_MINUS_I)]
                pacc = ps.tile([65, 1024], F32, name=f"pa{it}", tag="acc",
                               bufs=1)
                pending = None
                for idx, j in enumerate(js):
                    j0 = j * 128
                    fold = hh == 0 and i0 - j0 >= FOLD_I_MINUS_J
                    pS = ps.tile([128, 1024], F32, name=f"pS{it}_{j}", tag="sc")
                    if hh == 0:
                        kT = qkvT[:, 1, b * 2048 + j0: b * 2048 + j0 + 128]
                    else:
                        kT = kpadB[:, b * 2048 + j0: b * 2048 + j0 + 128]
                    for iq in range(2):
                        ii = i0 + iq * 512
                        sl = pS[:, iq * 512:(iq + 1) * 512]
                        qT = qkvT[:, 0, b * 2048 + ii: b * 2048 + ii + 512]
                        nc.tensor.matmul(sl, kT, qT, start=True, stop=fold)
                        if not fold:
                            c0 = ii - j0 + (S - 128)
                            if hh == 0:
                                rhs = btab0[:, c0 - BT0_OFF:c0 - BT0_OFF + 512]
                            else:
                                rhs = btab1[:, c0:c0 + 512]
                            nc.tensor.matmul(sl, identr[:], rhs,
                                             start=False, stop=True)
                    pb = ppool.tile([128, 1024], F16, name=f"pb{it}_{j}",
                                    tag="pb")
                    nc.scalar.activation(pb[:], pS[:], Exp,
                                         bias=(zero0 if fold else neg8)[:, 0:1],
                                         scale=1.0)
                    if pending is not None:
                        pvb, pvj, first = pending
                        for iq in range(2):
                            nc.tensor.matmul(pacc[:, iq * 512:(iq + 1) * 512],
                                             v_nat[:, b * 16 + pvj, hh, :],
                                             pvb[:, iq * 512:(iq + 1) * 512],
                                             start=first, stop=False)
                    pending = (pb, j, idx == 0)
                pvb, pvj, first = pending
                for iq in range(2):
                    nc.tensor.matmul(pacc[:, iq * 512:(iq + 1) * 512],
                                     v_nat[:, b * 16 + pvj, hh, :],
                                     pvb[:, iq * 512:(iq + 1) * 512],
                                     start=first, stop=True)
                # normalization: oT = pacc[0:64] * (1/rowsum).
                # reciprocal runs in [8,128] layout (cheap); row<->col reshapes
                # ride on DMA; the broadcast runs on the idle GpSimd engine.
                sumr = wk.tile([1, 1024], F32, name=f"sr{it}", tag="sumr",
                               bufs=1)
                nc.vector.tensor_copy(sumr[:], pacc[64:65, :])
                sumc = wk.tile([8, 128], F32, name=f"sc{it}", tag="sumc")
                nc.sync.dma_start(sumc[:],
                                  sumr[:].rearrange("o (p a) -> o p a", a=128))
                inv8 = wk.tile([8, 128], F32, name=f"i8{it}", tag="inv8")
                nc.vector.reciprocal(inv8[:], sumc[:])
                invr = wk.tile([1, 1024], F32, name=f"iv{it}", tag="invr",
                               bufs=1)
                nc.sync.dma_start(invr[:].rearrange("o (p a) -> o p a", a=128),
                                  inv8[:])
                invbc = wk.tile([128, 1024], F32, name=f"ib{it}", tag="invbc",
                                bufs=1)
                nc.gpsimd.partition_broadcast(invbc[:], invr[:], channels=128)
                osl = oT[hb:hb + 64, b * 2048 + i0: b * 2048 + i0 + 1024]
                with nc.allow_low_precision(reason="f32r out"):
                    nc.vector.tensor_copy(osl, pacc[0:64, :])
                    nc.vector.tensor_tensor(osl, osl, invbc[hb:hb + 64, :], MUL)

            def out_proj(b, ih):
                for tloc in range(8):
                    tb = b * 16 + ih * 8 + tloc
                    for cq in range(2):
                        py_ = ps.tile([128, 512], F32, name=f"py{tb}_{cq}",
                                      tag="py")
                        nc.tensor.matmul(py_[:],
                                         oT[:, tb * 128:(tb + 1) * 128],
                                         wo_sb[:, cq * 512:(cq + 1) * 512],
                                         start=True, stop=True)
                        ytile = wk.tile([128, 512], F32, name=f"yt{tb}_{cq}",
                                        tag="ytile")
                        if cq == 0:
                            nc.vector.tensor_copy(ytile[:], py_[:])
                        else:
                            nc.scalar.copy(ytile[:], py_[:])
                        nc.sync.dma_start(y_r[tb][:, cq * 512:(cq + 1) * 512],
                                          ytile[:])

            in_proj(0)
            load_tables()
            v_transpose(0)
            attn_iter(0, 0, 0)
            attn_iter(0, 0, 1)
            attn_iter(0, 1, 0)
            attn_iter(0, 1, 1)
            in_proj(1)
            v_transpose(1)
            attn_iter(1, 0, 0)
            out_proj(0, 0)
            attn_iter(1, 0, 1)
            out_proj(0, 1)
            attn_iter(1, 1, 0)
            out_proj(1, 0)
            attn_iter(1, 1, 1)
            out_proj(1, 1)

    nc.compile()
    return nc


def _make_inmaps(x, in_proj_weight, in_proj_bias, out_proj_weight):
    slopes = _slopes()
    xT = np.ascontiguousarray(
        x.reshape(TOK, C).T.astype(np.float32))  # [C, TOK]

    in_maps = []
    p = np.arange(128, dtype=np.float64)[:, None]
    cc = np.arange(BTW, dtype=np.float64)[None, :]
    for c in range(NCORE):
        heads = (c, c + 8)
        rows = []
        for sec in range(3):  # q, k, v
            for h in heads:
                rows.extend(range(sec * C + h * D, sec * C + (h + 1) * D))
        rows = np.array(rows)
        wq = in_proj_weight[rows, :].astype(np.float32).copy()
        bq = in_proj_bias[rows].astype(np.float32).copy()
        wq[:128] *= SCALE  # fold q scaling
        bq[:128] *= SCALE
        wqkvt = np.ascontiguousarray(wq.T)  # [C, 384]
        bqkv = np.ascontiguousarray(bq.reshape(3, 128).T)  # [128, 3]

        btarr = np.empty((2, 128, BTW), dtype=np.float32)
        for hh, h in enumerate(heads):
            btarr[hh] = np.minimum(
                float(slopes[h]) * (cc - (S - 128) - p), float(MAX_BIAS)
            ).astype(np.float32)

        ocols = np.array(
            [heads[0] * D + d for d in range(D)]
            + [heads[1] * D + d for d in range(D)]
        )
        wotr = np.ascontiguousarray(
            out_proj_weight[:, ocols].T.astype(np.float32))  # [128, C]

        in_maps.append({
            "xt": xT,
            "wqkvt": wqkvt,
            "bqkv": bqkv,
            "bt": btarr,
            "wot": wotr,
        })
    return in_maps


def run(inputs: dict, trace: bool = False):
    from concourse.bass_utils import run_bass_kernel_spmd

    nc = _program()
    in_maps = _make_inmaps(
        np.asarray(inputs["x"]),
        np.asarray(inputs["in_proj_weight"]),
        np.asarray(inputs["in_proj_bias"]),
        np.asarray(inputs["out_proj_weight"]),
    )
    res = run_bass_kernel_spmd(nc, in_maps, list(range(NCORE)), trace=trace)
    acc = np.zeros((TOK, C), dtype=np.float64)
    for r in res.results:
        acc += r["y"].astype(np.float64)
    acc += np.asarray(inputs["out_proj_bias"]).astype(np.float64)[None, :]
    out = acc.astype(np.float32).reshape(B, S, C)
    return out, res


def kernel(**inputs) -> np.ndarray:
    return run(inputs, trace=False)[0]


# revision 17
# speedup vs baseline: 1.1444x; 1.0424x over previous
"""ALiBi attention (B=2, S=2048, C=1024, H=16) on 8 trn2 NeuronCores.

Sharding: head-parallel. Core c owns heads (c, c+8) for both batches:
  - in_proj computed per-core only for its 6 head-slices (q,k,v x 2 heads),
    directly in transposed [channel, token] layout (x is host-transposed).
  - scores are computed transposed (S^T[j,i] = k_j . q_i) so softmax j-sums
    come from a ones-column augmented onto v, and the probability matrix is
    never transposed.
  - ALiBi bias min(slope*(i-j), 8) is injected into the score PSUM with an
    identity matmul against a host-precomputed shifted bias table; tiles where
    the bias is saturated at +8 skip the inject (the +8 cancels against the
    exp's -8 range shift), and far-future tiles with negligible probability
    mass are skipped entirely. Both classifications depend only on the head
    SLOT (slot 0 = heads 0..7, slot 1 = heads 8..15), so the single SPMD
    program stays valid on every core.
  - k stationaries are zero-padded to K=128 per head (the other head's rows
    are 0, killing its q rows in the shared moving operand): mixed K=64/K=128
    f32r matmul streams reconfigure the PE array and run ~3x slower.
  - out_proj is row-parallel: each core emits a partial y; the host sums the
    8 partials and adds out_proj_bias (the "all-reduce").
"""
import functools
import math
import sys

sys.path.insert(0, "/opt/trn_rl_repo")

import numpy as np

B, S, C, H, D = 2, 2048, 1024, 16, 64
TOK = B * S
NCORE = 8
MAX_BIAS = 8.0
BTW = 2 * S - 128       # shifted bias-table width (full, for slot-1 heads)
BT0_OFF = 384           # slot-0 table column offset (unfolded tiles only)
BT0_W = 2816            # slot-0 table width
SCALE = float(D) ** -0.5
SKIP_J_MINUS_I = 1483   # skip tile if j0 - i0 >= this (slot 0 only)
FOLD_I_MINUS_J = 255    # inject-free tile if i0 - j0 >= this (slot 0 only)


def _slopes() -> np.ndarray:
    start = 2.0 ** (-(2.0 ** (-(math.log2(H) - 3))))
    return np.array([start * start**i for i in range(H)], dtype=np.float32)


@functools.lru_cache(maxsize=1)
def _program():
    import concourse.mybir as mybir
    import concourse.tile as tile
    from concourse import bacc
    from concourse.masks import make_identity

    F32 = mybir.dt.float32
    F32R = mybir.dt.float32r
    F16 = mybir.dt.float16  # noqa
    BF16 = mybir.dt.bfloat16
    U32 = mybir.dt.uint32
    Exp = mybir.ActivationFunctionType.Exp
    MUL = mybir.AluOpType.mult

    nc = bacc.Bacc("TRN2", target_bir_lowering=False, debug=False)

    xt = nc.dram_tensor("xt", [C, TOK], F32R, kind="ExternalInput").ap()
    wqkvt = nc.dram_tensor("wqkvt", [C, 384], F32R, kind="ExternalInput").ap()
    bqkv = nc.dram_tensor("bqkv", [128, 3], F32, kind="ExternalInput").ap()
    bt = nc.dram_tensor("bt", [2, 128, BTW], F32R, kind="ExternalInput").ap()
    wot = nc.dram_tensor("wot", [128, C], F32R, kind="ExternalInput").ap()
    y = nc.dram_tensor("y", [TOK, C], F32, kind="ExternalOutput").ap()

    with tile.TileContext(nc) as tc:
        with tc.tile_pool(name="const", bufs=1) as cpool, \
             tc.tile_pool(name="wpool", bufs=1) as wpool, \
             tc.tile_pool(name="qkvp", bufs=1) as qkvp, \
             tc.tile_pool(name="xin", bufs=2) as xpool, \
             tc.tile_pool(name="probs", bufs=2) as ppool, \
             tc.tile_pool(name="work", bufs=2) as wk, \
             tc.tile_pool(name="ps", bufs=2, space="PSUM") as ps:

            ident = cpool.tile([128, 128], F32, name="ident")
            make_identity(nc, ident[:])
            identr = cpool.tile([128, 128], F32R, name="identr")
            nc.vector.tensor_copy(identr[:], ident[:])
            neg8 = cpool.tile([128, 1], F32, name="neg8")
            nc.vector.memset(neg8[:], -MAX_BIAS)
            zero0 = cpool.tile([128, 1], F32, name="zero0")
            nc.vector.memset(zero0[:], 0.0)
            heat = cpool.tile([128, 128], BF16, name="heat")
            nc.vector.tensor_copy(heat[:], ident[:])

            wq_sb = wpool.tile([128, 8, 384], F32R, name="wq_sb")
            nc.sync.dma_start(wq_sb[:],
                              wqkvt.rearrange("(co p) n -> p co n", p=128))
            bq_sb = wpool.tile([128, 3], F32, name="bq_sb")
            nc.sync.dma_start(bq_sb[:], bqkv)
            btab1 = wpool.tile([128, BTW], F32R, name="btab1")
            btab0 = wpool.tile([128, BT0_W], F32R, name="btab0")
            wo_sb = wpool.tile([128, C], F32R, name="wo_sb")

            def load_tables():
                nc.sync.dma_start(btab1[:],
                                  bt.rearrange("h p c -> p h c")[:, 1, :])
                nc.sync.dma_start(
                    btab0[:],
                    bt.rearrange("h p c -> p h c")[:, 0,
                                                   BT0_OFF:BT0_OFF + BT0_W])
                nc.sync.dma_start(wo_sb[:], wot)

            qkvT = qkvp.tile([128, 3, TOK], F32R, name="qkvT")
            kpadB = qkvp.tile([128, TOK], F32R, name="kpadB")
            nc.vector.memset(qkvT[64:128, 1, :].bitcast(U32), 0)
            nc.vector.memset(kpadB[0:64, :].bitcast(U32), 0)
            v_nat = qkvp.tile([128, 32, 2, 65], F16, name="v_nat")
            nc.vector.memset(v_nat[:, :, :, 64:65], 1.0)
            oT = qkvp.tile([128, TOK], F32R, name="oT")

            xt_r = xt.rearrange("(co p) t -> p co t", p=128)
            y_r = y.rearrange("(tb p) c -> tb p c", p=128)

            def in_proj(bb):
                for tb in range(4 * bb, 4 * bb + 4):
                    xtile = xpool.tile([128, 8, 512], F32R, name=f"xt{tb}",
                                       tag="xtile")
                    for cb in range(8):
                        nc.sync.dma_start(
                            xtile[:, cb:cb + 1, :],
                            xt_r[:, cb:cb + 1, tb * 512:(tb + 1) * 512])
                    for chb in range(3):
                        pin = ps.tile([128, 512], F32, name=f"pin{tb}_{chb}",
                                      tag="sc")
                        for cb in range(8):
                            nc.tensor.matmul(
                                pin[:],
                                wq_sb[:, cb, chb * 128:(chb + 1) * 128],
                                xtile[:, cb, :],
                                start=(cb == 0), stop=(cb == 7))
                        ts = slice(tb * 512, (tb + 1) * 512)
                        if chb == 1:
                            nc.vector.tensor_scalar_add(
                                qkvT[0:64, 1, ts], pin[0:64], bq_sb[0:64, 1:2])
                            nc.vector.tensor_scalar_add(
                                kpadB[64:128, ts], pin[64:128],
                                bq_sb[64:128, 1:2])
                        else:
                            nc.vector.tensor_scalar_add(
                                qkvT[:, chb, ts], pin[:], bq_sb[:, chb:chb + 1])

            def v_transpose(bb):
                for t32 in range(16 * bb, 16 * bb + 16):
                    pv = ps.tile([128, 128], F32, name=f"pv{t32}", tag="sc")
                    nc.tensor.transpose(
                        pv[:],
                        qkvT[:, 2, t32 * 128:(t32 + 1) * 128].bitcast(F32),
                        ident[:])
                    for hh in range(2):
                        nc.vector.tensor_copy(v_nat[:, t32, hh, 0:64],
                                              pv[:, hh * 64:hh * 64 + 64])

            def attn_iter(b, ih, hh):
                hb = hh * 64
                i0 = ih * 1024
                it = f"{b}{ih}{hh}"
                js = [j for j in range(16)
                      if not (hh == 0 and j * 128 - i0 >= SKIP_J_MINUS_I)]
                pacc = ps.tile([65, 1024], F32, name=f"pa{it}", tag="acc",
                               bufs=1)
                pending = None
                for idx, j in enumerate(js):
                    j0 = j * 128
                    fold = hh == 0 and i0 - j0 >= FOLD_I_MINUS_J
                    pS = ps.tile([128, 1024], F32, name=f"pS{it}_{j}", tag="sc")
                    if hh == 0:
                        kT = qkvT[:, 1, b * 2048 + j0: b * 2048 + j0 + 128]
                    else:
                        kT = kpadB[:, b * 2048 + j0: b * 2048 + j0 + 128]
                    for iq in range(2):
                        ii = i0 + iq * 512
                        sl = pS[:, iq * 512:(iq + 1) * 512]
                        qT = qkvT[:, 0, b * 2048 + ii: b * 2048 + ii + 512]
                        nc.tensor.matmul(sl, kT, qT, start=True, stop=fold)
                        if not fold:
                            c0 = ii - j0 + (S - 128)
                            if hh == 0:
                                rhs = btab0[:, c0 - BT0_OFF:c0 - BT0_OFF + 512]
                            else:
                                rhs = btab1[:, c0:c0 + 512]
                            nc.tensor.matmul(sl, identr[:], rhs,
                                             start=False, stop=True)
                    pb = ppool.tile([128, 1024], F16, name=f"pb{it}_{j}",
                                    tag="pb")
                    nc.scalar.activation(pb[:], pS[:], Exp,
                                         bias=(zero0 if fold else neg8)[:, 0:1],
                                         scale=1.0)
                    if pending is not None:
                        pvb, pvj, first = pending
                        for iq in range(2):
                            nc.tensor.matmul(pacc[:, iq * 512:(iq + 1) * 512],
                                             v_nat[:, b * 16 + pvj, hh, :],
                                             pvb[:, iq * 512:(iq + 1) * 512],
                                             start=first, stop=False)
                    pending = (pb, j, idx == 0)
                pvb, pvj, first = pending
                for iq in range(2):
                    nc.tensor.matmul(pacc[:, iq * 512:(iq + 1) * 512],
                                     v_nat[:, b * 16 + pvj, hh, :],
                                     pvb[:, iq * 512:(iq + 1) * 512],
                                     start=first, stop=True)
                # normalization: oT = pacc[0:64] * (1/rowsum).
                # reciprocal runs in [8,128] layout (cheap); row<->col reshapes
                # ride on DMA; the broadcast runs on the idle GpSimd engine.
                sumr = wk.tile([1, 1024], F32, name=f"sr{it}", tag="sumr",
                               bufs=1)
                nc.vector.tensor_copy(sumr[:], pacc[64:65, :])
                sumc = wk.tile([8, 128], F32, name=f"sc{it}", tag="sumc")
                nc.sync.dma_start(sumc[:],
                                  sumr[:].rearrange("o (p a) -> o p a", a=128))
                inv8 = wk.tile([8, 128], F32, name=f"i8{it}", tag="inv8")
                nc.vector.reciprocal(inv8[:], sumc[:])
                invr = wk.tile([1, 1024], F32, name=f"iv{it}", tag="invr",
                               bufs=1)
                nc.sync.dma_start(invr[:].rearrange("o (p a) -> o p a", a=128),
                                  inv8[:])
                invbc = wk.tile([128, 1024], F32, name=f"ib{it}", tag="invbc",
                                bufs=1)
                nc.gpsimd.partition_broadcast(invbc[:], invr[:], channels=128)
                osl = oT[hb:hb + 64, b * 2048 + i0: b * 2048 + i0 + 1024]
                with nc.allow_low_precision(reason="f32r out"):
                    nc.vector.tensor_copy(osl, pacc[0:64, :])
                    nc.vector.tensor_tensor(osl, osl, invbc[hb:hb + 64, :], MUL)

            def out_proj(b, ih):
                for tloc in range(8):
                    tb = b * 16 + ih * 8 + tloc
                    for cq in range(2):
                        py_ = ps.tile([128, 512], F32, name=f"py{tb}_{cq}",
                                      tag="py")
                        nc.tensor.matmul(py_[:],
                                         oT[:, tb * 128:(tb + 1) * 128],
                                         wo_sb[:, cq * 512:(cq + 1) * 512],
                                         start=True, stop=True)
                        ytile = wk.tile([128, 512], F32, name=f"yt{tb}_{cq}",
                                        tag="ytile", bufs=4)
                        nc.vector.tensor_copy(ytile[:], py_[:])
                        nc.sync.dma_start(y_r[tb][:, cq * 512:(cq + 1) * 512],
                                          ytile[:])

            in_proj(0)
            load_tables()
            v_transpose(0)
            attn_iter(0, 0, 0)
            attn_iter(0, 0, 1)
            attn_iter(0, 1, 0)
            attn_iter(0, 1, 1)
            in_proj(1)
            v_transpose(1)
            attn_iter(1, 0, 0)
            out_proj(0, 0)
            attn_iter(1, 0, 1)
            out_proj(0, 1)
            attn_iter(1, 1, 0)
            out_proj(1, 0)
            attn_iter(1, 1, 1)
            out_proj(1, 1)

    nc.compile()
    return nc


def _make_inmaps(x, in_proj_weight, in_proj_bias, out_proj_weight):
    slopes = _slopes()
    xT = np.ascontiguousarray(
        x.reshape(TOK, C).T.astype(np.float32))  # [C, TOK]

    in_maps = []
    p = np.arange(128, dtype=np.float64)[:, None]
    cc = np.arange(BTW, dtype=np.float64)[None, :]
    for c in range(NCORE):
        heads = (c, c + 8)
        rows = []
        for sec in range(3):  # q, k, v
            for h in heads:
                rows.extend(range(sec * C + h * D, sec * C + (h + 1) * D))
        rows = np.array(rows)
        wq = in_proj_weight[rows, :].astype(np.float32).copy()
        bq = in_proj_bias[rows].astype(np.float32).copy()
        wq[:128] *= SCALE  # fold q scaling
        bq[:128] *= SCALE
        wqkvt = np.ascontiguousarray(wq.T)  # [C, 384]
        bqkv = np.ascontiguousarray(bq.reshape(3, 128).T)  # [128, 3]

        btarr = np.empty((2, 128, BTW), dtype=np.float32)
        for hh, h in enumerate(heads):
            btarr[hh] = np.minimum(
                float(slopes[h]) * (cc - (S - 128) - p), float(MAX_BIAS)
            ).astype(np.float32)

        ocols = np.array(
            [heads[0] * D + d for d in range(D)]
            + [heads[1] * D + d for d in range(D)]
        )
        wotr = np.ascontiguousarray(
            out_proj_weight[:, ocols].T.astype(np.float32))  # [128, C]

        in_maps.append({
            "xt": xT,
            "wqkvt": wqkvt,
            "bqkv": bqkv,
            "bt": btarr,
            "wot": wotr,
        })
    return in_maps


def run(inputs: dict, trace: bool = False):
    from concourse.bass_utils import run_bass_kernel_spmd

    nc = _program()
    in_maps = _make_inmaps(
        np.asarray(inputs["x"]),
        np.asarray(inputs["in_proj_weight"]),
        np.asarray(inputs["in_proj_bias"]),
        np.asarray(inputs["out_proj_weight"]),
    )
    res = run_bass_kernel_spmd(nc, in_maps, list(range(NCORE)), trace=trace)
    acc = np.zeros((TOK, C), dtype=np.float64)
    for r in res.results:
        acc += r["y"].astype(np.float64)
    acc += np.asarray(inputs["out_proj_bias"]).astype(np.float64)[None, :]
    out = acc.astype(np.float32).reshape(B, S, C)
    return out, res


def kernel(**inputs) -> np.ndarray:
    return run(inputs, trace=False)[0]


# revision 21
# speedup vs baseline: 1.2528x; 1.0947x over previous
"""ALiBi attention (B=2, S=2048, C=1024, H=16) on 8 trn2 NeuronCores.

Sharding: head-parallel. Core c owns heads (c, c+8) for both batches:
  - in_proj computed per-core only for its 6 head-slices (q,k,v x 2 heads),
    directly in transposed [channel, token] layout (x is host-transposed).
  - scores are computed transposed (S^T[j,i] = k_j . q_i) so softmax j-sums
    come from a ones-column augmented onto v, and the probability matrix is
    never transposed.
  - ALiBi bias min(slope*(i-j), 8) is injected into the score PSUM with an
    identity matmul against a host-precomputed shifted bias table; tiles where
    the bias is saturated at +8 skip the inject (the +8 cancels against the
    exp's -8 range shift), and far-future tiles with negligible probability
    mass are skipped entirely. Both classifications depend only on the head
    SLOT (slot 0 = heads 0..7, slot 1 = heads 8..15), so the single SPMD
    program stays valid on every core.
  - k stationaries are zero-padded to K=128 per head (the other head's rows
    are 0, killing its q rows in the shared moving operand): mixed K=64/K=128
    f32r matmul streams reconfigure the PE array and run ~3x slower.
  - out_proj is row-parallel: each core emits a partial y; the host sums the
    8 partials and adds out_proj_bias (the "all-reduce").
"""
import functools
import math
import sys

sys.path.insert(0, "/opt/trn_rl_repo")

import numpy as np

B, S, C, H, D = 2, 2048, 1024, 16, 64
TOK = B * S
NCORE = 8
MAX_BIAS = 8.0
BTW = 2 * S - 128       # shifted bias-table width (full, for slot-1 heads)
BT0_OFF = 384           # slot-0 table column offset (unfolded tiles only)
BT0_W = 2816            # slot-0 table width
SCALE = float(D) ** -0.5
SKIP_J_MINUS_I = 1483   # skip tile if j0 - i0 >= this (slot 0 only)
FOLD_I_MINUS_J = 255    # inject-free tile if i0 - j0 >= this (slot 0 only)


def _slopes() -> np.ndarray:
    start = 2.0 ** (-(2.0 ** (-(math.log2(H) - 3))))
    return np.array([start * start**i for i in range(H)], dtype=np.float32)


@functools.lru_cache(maxsize=1)
def _program():
    import concourse.mybir as mybir
    import concourse.tile as tile
    from concourse import bacc
    from concourse.masks import make_identity

    F32 = mybir.dt.float32
    F32R = mybir.dt.float32r
    F16 = mybir.dt.float16  # noqa
    BF16 = mybir.dt.bfloat16
    U32 = mybir.dt.uint32
    Exp = mybir.ActivationFunctionType.Exp
    MUL = mybir.AluOpType.mult

    nc = bacc.Bacc("TRN2", target_bir_lowering=False, debug=False)

    xt = nc.dram_tensor("xt", [C, TOK], F32R, kind="ExternalInput").ap()
    wqkvt = nc.dram_tensor("wqkvt", [C, 384], F32R, kind="ExternalInput").ap()
    bqkv = nc.dram_tensor("bqkv", [128, 3], F32, kind="ExternalInput").ap()
    bt = nc.dram_tensor("bt", [2, 128, BTW], F16, kind="ExternalInput").ap()
    wot = nc.dram_tensor("wot", [128, C], F32R, kind="ExternalInput").ap()
    y = nc.dram_tensor("y", [TOK, C], F32, kind="ExternalOutput").ap()

    with tile.TileContext(nc) as tc:
        with tc.tile_pool(name="const", bufs=1) as cpool, \
             tc.tile_pool(name="wpool", bufs=1) as wpool, \
             tc.tile_pool(name="qkvp", bufs=1) as qkvp, \
             tc.tile_pool(name="xin", bufs=2) as xpool, \
             tc.tile_pool(name="probs", bufs=2) as ppool, \
             tc.tile_pool(name="work", bufs=2) as wk, \
             tc.tile_pool(name="ps", bufs=2, space="PSUM") as ps:

            ident = cpool.tile([128, 128], F32, name="ident")
            make_identity(nc, ident[:])
            identr = cpool.tile([128, 128], F32R, name="identr")
            nc.vector.tensor_copy(identr[:], ident[:])
            neg8 = cpool.tile([128, 1], F32, name="neg8")
            nc.vector.memset(neg8[:], -MAX_BIAS)
            zero0 = cpool.tile([128, 1], F32, name="zero0")
            nc.vector.memset(zero0[:], 0.0)
            heat = cpool.tile([128, 128], BF16, name="heat")
            nc.vector.tensor_copy(heat[:], ident[:])

            wq_sb = wpool.tile([128, 8, 384], F32R, name="wq_sb")
            nc.sync.dma_start(wq_sb[:],
                              wqkvt.rearrange("(co p) n -> p co n", p=128))
            bq_sb = wpool.tile([128, 3], F32, name="bq_sb")
            nc.sync.dma_start(bq_sb[:], bqkv)
            btab1 = wpool.tile([128, BTW], F16, name="btab1")
            btab0 = wpool.tile([128, BT0_W], F16, name="btab0")
            wo_sb = wpool.tile([128, C], F32R, name="wo_sb")

            def load_tables():
                nc.sync.dma_start(btab1[:],
                                  bt.rearrange("h p c -> p h c")[:, 1, :])
                nc.sync.dma_start(
                    btab0[:],
                    bt.rearrange("h p c -> p h c")[:, 0,
                                                   BT0_OFF:BT0_OFF + BT0_W])
                nc.sync.dma_start(wo_sb[:], wot)

            qkvT = qkvp.tile([128, 3, TOK], F32R, name="qkvT")
            kpadB = qkvp.tile([128, TOK], F32R, name="kpadB")
            nc.vector.memset(qkvT[64:128, 1, :].bitcast(U32), 0)
            nc.vector.memset(kpadB[0:64, :].bitcast(U32), 0)
            v_nat = qkvp.tile([128, 32, 2, 65], F16, name="v_nat")
            nc.vector.memset(v_nat[:, :, :, 64:65], 1.0)
            oT = qkvp.tile([128, TOK], F32R, name="oT")

            xt_r = xt.rearrange("(co p) t -> p co t", p=128)
            y_r = y.rearrange("(tb p) c -> tb p c", p=128)

            def in_proj(bb):
                for tb in range(4 * bb, 4 * bb + 4):
                    xtile = xpool.tile([128, 8, 512], F32R, name=f"xt{tb}",
                                       tag="xtile")
                    for cb in range(8):
                        nc.sync.dma_start(
                            xtile[:, cb:cb + 1, :],
                            xt_r[:, cb:cb + 1, tb * 512:(tb + 1) * 512])
                    for chb in range(3):
                        pin = ps.tile([128, 512], F32, name=f"pin{tb}_{chb}",
                                      tag="sc")
                        for cb in range(8):
                            nc.tensor.matmul(
                                pin[:],
                                wq_sb[:, cb, chb * 128:(chb + 1) * 128],
                                xtile[:, cb, :],
                                start=(cb == 0), stop=(cb == 7))
                        ts = slice(tb * 512, (tb + 1) * 512)
                        if chb == 1:
                            nc.vector.tensor_scalar_add(
                                qkvT[0:64, 1, ts], pin[0:64], bq_sb[0:64, 1:2])
                            nc.vector.tensor_scalar_add(
                                kpadB[64:128, ts], pin[64:128],
                                bq_sb[64:128, 1:2])
                        else:
                            nc.vector.tensor_scalar_add(
                                qkvT[:, chb, ts], pin[:], bq_sb[:, chb:chb + 1])

            def v_transpose(bb):
                for t32 in range(16 * bb, 16 * bb + 16):
                    pv = ps.tile([128, 128], F32, name=f"pv{t32}", tag="sc")
                    nc.tensor.transpose(
                        pv[:],
                        qkvT[:, 2, t32 * 128:(t32 + 1) * 128].bitcast(F32),
                        ident[:])
                    for hh in range(2):
                        nc.vector.tensor_copy(v_nat[:, t32, hh, 0:64],
                                              pv[:, hh * 64:hh * 64 + 64])

            def attn_iter(b, ih, hh):
                hb = hh * 64
                i0 = ih * 1024
                it = f"{b}{ih}{hh}"
                js = [j for j in range(16)
                      if not (hh == 0 and j * 128 - i0 >= SKIP_J_MINUS_I)]
                pacc = ps.tile([65, 1024], F32, name=f"pa{it}", tag="acc",
                               bufs=1)
                pend = []  # PV queue, depth 2 hides the exp->EB-mult chain

                def flush_pv(last):
                    pvb, pvj, first = pend.pop(0)
                    for iq in range(2):
                        nc.tensor.matmul(pacc[:, iq * 512:(iq + 1) * 512],
                                         v_nat[:, b * 16 + pvj, hh, :],
                                         pvb[:, iq * 512:(iq + 1) * 512],
                                         start=first, stop=last)

                for idx, j in enumerate(js):
                    j0 = j * 128
                    fold = hh == 0 and i0 - j0 >= FOLD_I_MINUS_J
                    pS = ps.tile([128, 1024], F32, name=f"pS{it}_{j}", tag="sc")
                    if hh == 0:
                        kT = qkvT[:, 1, b * 2048 + j0: b * 2048 + j0 + 128]
                    else:
                        kT = kpadB[:, b * 2048 + j0: b * 2048 + j0 + 128]
                    for iq in range(2):
                        ii = i0 + iq * 512
                        sl = pS[:, iq * 512:(iq + 1) * 512]
                        qT = qkvT[:, 0, b * 2048 + ii: b * 2048 + ii + 512]
                        nc.tensor.matmul(sl, kT, qT, start=True, stop=True)
                    # probs = exp(s) * exp(bias-8): same value range as the
                    # additive exp(s+bias-8); the fp16 table multiply runs on
                    # the underloaded DVE instead of PE identity-injects.
                    pb = ppool.tile([128, 1024], F16, name=f"pb{it}_{j}",
                                    tag="pb", bufs=4)
                    nc.scalar.activation(pb[:], pS[:], Exp,
                                         bias=zero0[:, 0:1], scale=1.0)
                    if not fold:
                        c0 = i0 - j0 + (S - 128)
                        if hh == 0:
                            eb = btab0[:, c0 - BT0_OFF:c0 - BT0_OFF + 1024]
                        else:
                            eb = btab1[:, c0:c0 + 1024]
                        with nc.allow_low_precision(reason="fp16 probs"):
                            nc.vector.tensor_tensor(pb[:], pb[:], eb, MUL)
                    if len(pend) == 2:
                        flush_pv(False)
                    pend.append((pb, j, idx == 0))
                while pend:
                    flush_pv(len(pend) == 1)
                # normalization: oT = pacc[0:64] * (1/rowsum).
                # reciprocal runs in [8,128] layout (cheap); row<->col reshapes
                # ride on DMA; the broadcast runs on the idle GpSimd engine.
                sumr = wk.tile([1, 1024], F32, name=f"sr{it}", tag="sumr",
                               bufs=1)
                nc.vector.tensor_copy(sumr[:], pacc[64:65, :])
                sumc = wk.tile([8, 128], F32, name=f"sc{it}", tag="sumc")
                nc.sync.dma_start(sumc[:],
                                  sumr[:].rearrange("o (p a) -> o p a", a=128))
                inv8 = wk.tile([8, 128], F32, name=f"i8{it}", tag="inv8")
                nc.vector.reciprocal(inv8[:], sumc[:])
                invr = wk.tile([1, 1024], F32, name=f"iv{it}", tag="invr",
                               bufs=1)
                nc.sync.dma_start(invr[:].rearrange("o (p a) -> o p a", a=128),
                                  inv8[:])
                invbc = wk.tile([128, 1024], F32, name=f"ib{it}", tag="invbc",
                                bufs=1)
                nc.gpsimd.partition_broadcast(invbc[:], invr[:], channels=128)
                osl = oT[hb:hb + 64, b * 2048 + i0: b * 2048 + i0 + 1024]
                with nc.allow_low_precision(reason="f32r out"):
                    nc.vector.tensor_copy(osl, pacc[0:64, :])
                    nc.vector.tensor_tensor(osl, osl, invbc[hb:hb + 64, :], MUL)

            def out_proj(b, ih):
                for tloc in range(8):
                    tb = b * 16 + ih * 8 + tloc
                    for cq in range(2):
                        py_ = ps.tile([128, 512], F32, name=f"py{tb}_{cq}",
                                      tag="py")
                        nc.tensor.matmul(py_[:],
                                         oT[:, tb * 128:(tb + 1) * 128],
                                         wo_sb[:, cq * 512:(cq + 1) * 512],
                                         start=True, stop=True)
                        ytile = wk.tile([128, 512], F32, name=f"yt{tb}_{cq}",
                                        tag="ytile", bufs=4)
                        nc.vector.tensor_copy(ytile[:], py_[:])
                        nc.sync.dma_start(y_r[tb][:, cq * 512:(cq + 1) * 512],
                                          ytile[:])

            in_proj(0)
            load_tables()
            v_transpose(0)
            attn_iter(0, 0, 0)
            attn_iter(0, 0, 1)
            attn_iter(0, 1, 0)
            attn_iter(0, 1, 1)
            in_proj(1)
            v_transpose(1)
            attn_iter(1, 0, 0)
            out_proj(0, 0)
            attn_iter(1, 0, 1)
            out_proj(0, 1)
            attn_iter(1, 1, 0)
            out_proj(1, 0)
            attn_iter(1, 1, 1)
            out_proj(1, 1)

    nc.compile()
    return nc


def _make_inmaps(x, in_proj_weight, in_proj_bias, out_proj_weight):
    slopes = _slopes()
    xT = np.ascontiguousarray(
        x.reshape(TOK, C).T.astype(np.float32))  # [C, TOK]

    in_maps = []
    p = np.arange(128, dtype=np.float64)[:, None]
    cc = np.arange(BTW, dtype=np.float64)[None, :]
    for c in range(NCORE):
        heads = (c, c + 8)
        rows = []
        for sec in range(3):  # q, k, v
            for h in heads:
                rows.extend(range(sec * C + h * D, sec * C + (h + 1) * D))
        rows = np.array(rows)
        wq = in_proj_weight[rows, :].astype(np.float32).copy()
        bq = in_proj_bias[rows].astype(np.float32).copy()
        wq[:128] *= SCALE  # fold q scaling
        bq[:128] *= SCALE
        wqkvt = np.ascontiguousarray(wq.T)  # [C, 384]
        bqkv = np.ascontiguousarray(bq.reshape(3, 128).T)  # [128, 3]

        btarr = np.empty((2, 128, BTW), dtype=np.float16)
        for hh, h in enumerate(heads):
            bias = np.minimum(float(slopes[h]) * (cc - (S - 128) - p),
                              float(MAX_BIAS))
            btarr[hh] = np.exp(bias - float(MAX_BIAS)).astype(np.float16)

        ocols = np.array(
            [heads[0] * D + d for d in range(D)]
            + [heads[1] * D + d for d in range(D)]
        )
        wotr = np.ascontiguousarray(
            out_proj_weight[:, ocols].T.astype(np.float32))  # [128, C]

        in_maps.append({
            "xt": xT,
            "wqkvt": wqkvt,
            "bqkv": bqkv,
            "bt": btarr,
            "wot": wotr,
        })
    return in_maps


def run(inputs: dict, trace: bool = False):
    from concourse.bass_utils import run_bass_kernel_spmd

    nc = _program()
    in_maps = _make_inmaps(
        np.asarray(inputs["x"]),
        np.asarray(inputs["in_proj_weight"]),
        np.asarray(inputs["in_proj_bias"]),
        np.asarray(inputs["out_proj_weight"]),
    )
    res = run_bass_kernel_spmd(nc, in_maps, list(range(NCORE)), trace=trace)
    acc = np.zeros((TOK, C), dtype=np.float64)
    for r in res.results:
        acc += r["y"].astype(np.float64)
    acc += np.asarray(inputs["out_proj_bias"]).astype(np.float64)[None, :]
    out = acc.astype(np.float32).reshape(B, S, C)
    return out, res


def kernel(**inputs) -> np.ndarray:
    return run(inputs, trace=False)[0]
